# revision 40
# baseline (speedup 1.0000x reference)
"""RGCN 2-layer + pair-MLP Trainium2 kernel (8 NeuronCores, SPMD).

v3: fp16 hot path, edges sharded by dst range (12500 nodes/core).

Layer 1 needs no on-device gather at all: the host pre-sorts x rows into
edge-stream order (x_stream = x16[src, :] in (dst-tile-group, relation,
dst-tile) order), so the device just streams it sequentially with plain
DMAs. Layer 2 gathers h1 rows per edge with SWDGE dma_gather (1024-idx
gathers round-robined over 4 SWDGE queues; int16 indices force 4 source
chunks). Both layers segment-sum into PSUM via narrow [128,128] one-hot
matmuls (fp16, relation-major emission so each PSUM bank has only one
open accumulation group), apply the mean reciprocal at PSUM eviction
(DVE tensor_tensor against a per-dst-tile recip tile), then run the
per-relation transforms + root + bias as fp16 matmuls. h1 moves between
cores as a fp16 AllGather; the final pair MLP uses the local-partial +
AllReduce trick.
"""
import sys
sys.path.insert(0, '/opt/trn_rl_repo')

import numpy as np
import concourse.bass as bass
import concourse.bacc as bacc
import concourse.tile as tile
import concourse.mybir as mybir
from concourse.bass_utils import run_bass_kernel_spmd
from concourse.masks import make_identity

F32 = mybir.dt.float32
F16 = mybir.dt.float16
I32 = mybir.dt.int32
I16 = mybir.dt.int16


class Cfg:
    def __init__(self, N, F, H, EMB, R, E, B, NC, CH=4, W=3, GL=1024, SC=4096):
        self.N, self.F, self.H, self.EMB, self.R, self.E, self.B = N, F, H, EMB, R, E, B
        self.NC = NC
        self.CH = CH                      # layer-2 src chunks (int16 idx)
        self.W = W                        # dst tiles per group
        self.GL = GL                      # idxs per dma_gather (layer 2)
        self.SC = SC                      # rows per stream DMA (layer 1)
        self.ND = N // NC
        self.DT = (self.ND + 127) // 128
        self.NDP = self.DT * 128
        self.CHS = N // CH
        self.DG = (self.DT + W - 1) // W
        self.S3 = self.R * 128


FULL = Cfg(N=100000, F=128, H=128, EMB=64, R=3, E=1600000, B=1024, NC=8)


def _common(cfg, edge_src, edge_dst, edge_type):
    N, NC, ND, R = cfg.N, cfg.NC, cfg.ND, cfg.R
    src = edge_src.astype(np.int64)
    dst = edge_dst.astype(np.int64)
    rel = edge_type.astype(np.int64)
    cnt = np.bincount(rel * N + dst, minlength=R * N).astype(np.float32)
    recip_tab = (1.0 / np.maximum(cnt, 1.0)).reshape(R, N)
    core = dst // ND
    dloc = dst - core * ND
    return src, dst, rel, core, dloc, recip_tab


def preprocess_l1(cfg, src, rel, core, dloc):
    """Layer 1: single host-sorted stream per core, cells (dg, r, w)."""
    NC, DT, R, W, DG, SC = cfg.NC, cfg.DT, cfg.R, cfg.W, cfg.DG, cfg.SC
    E = src.shape[0]
    d = dloc >> 7
    dg = d // W
    w = d - dg * W
    cell = (dg * R + rel) * W + w
    NCELL = DG * R * W
    kcell = core * NCELL + cell
    order = np.lexsort((src, kcell))
    src_s, core_s = src[order], core[order]
    dloc_s = dloc[order]
    kcell_s = kcell[order]

    cell_cnt = np.bincount(kcell, minlength=NC * NCELL)
    P = cell_cnt.reshape(NC, DG, R, W).max(axis=0)
    P[:, :, :] = np.maximum(P, 1)          # keep every (d, r) group alive
    for dd in range(DT, DG * W):
        P[dd // W, :, dd % W] = 0

    cell_off = np.zeros((DG, R, W), np.int64)
    stream_base = np.zeros(DG, np.int64)
    run = 0
    for gg in range(DG):
        stream_base[gg] = run
        acc = 0
        for r in range(R):
            for ww in range(W):
                cell_off[gg, r, ww] = acc
                acc += P[gg, r, ww]
        run += ((acc + 127) // 128) * 128
    TOT = int(run)
    grp_len = np.zeros(DG, np.int64)
    for gg in range(DG):
        nxt = stream_base[gg + 1] if gg + 1 < DG else TOT
        grp_len[gg] = nxt - stream_base[gg]

    cell_start = np.zeros(NC * NCELL + 1, np.int64)
    cell_start[1:] = np.cumsum(cell_cnt)
    rank = np.arange(E, dtype=np.int64) - cell_start[kcell_s]
    d_s = dloc_s >> 7
    dg_s = d_s // W
    w_s = d_s - dg_s * W
    rel_s2 = (kcell_s % NCELL // W) % R
    slot = stream_base[dg_s] + cell_off[dg_s, rel_s2, w_s] + rank

    seg_arr = np.full((NC, TOT), -1.0, np.float32)
    seg_arr[core_s, slot] = (dloc_s & 127).astype(np.float32)
    srcrow = np.zeros((NC, TOT), np.int64)
    srcrow[core_s, slot] = src_s

    # chunks: per group, sequential DMAs of <= SC rows (multiples of 128)
    chunks = []                      # (gg, clen, sbase)
    c_by_grp = [[] for _ in range(DG)]
    for gg in range(DG):
        L = int(grp_len[gg])
        o = 0
        while o < L:
            cl = min(SC, L - o)
            c_by_grp[gg].append(len(chunks))
            chunks.append((gg, cl, int(stream_base[gg]) + o))
            o += cl

    # items
    n_inc = np.zeros((DT, R), np.int64)
    tmp = []
    for co, (gg, cl, sb) in enumerate(chunks):
        for t in range(cl // 128):
            lo = (sb - int(stream_base[gg])) + t * 128
            hi = lo + 128
            for r in range(R):
                for ww in range(W):
                    dd = gg * W + ww
                    if dd >= DT or P[gg, r, ww] == 0:
                        continue
                    clo = int(cell_off[gg, r, ww])
                    chi = clo + int(P[gg, r, ww])
                    a, b = max(lo, clo), min(hi, chi)
                    if a >= b:
                        continue
                    tmp.append((co, t, dd, r, int(stream_base[gg]), lo, a, b))
                    n_inc[dd, r] += 1
    seen = np.zeros((DT, R), np.int64)
    items = []
    iseg = np.full((NC, 128, len(tmp)), -1.0, np.float32)
    for it_i, (co, t, dd, r, sb0, lo, a, b) in enumerate(tmp):
        seen[dd, r] += 1
        items.append((co, t, dd, r, seen[dd, r] == 1,
                      seen[dd, r] == n_inc[dd, r]))
        iseg[:, a - lo:b - lo, it_i] = seg_arr[:, sb0 + a:sb0 + b]
    return dict(chunks=chunks, c_by_grp=c_by_grp, items=items, TOT=TOT,
                NIT=len(items), srcrow=srcrow, iseg=iseg)


def preprocess_l2(cfg, src, rel, core, dloc):
    """Layer 2: SWDGE gathers, cells (dg, m, r, w), int16 chunked idx."""
    NC, DT, CH, CHS, R, W, GL, DG = (cfg.NC, cfg.DT, cfg.CH, cfg.CHS, cfg.R,
                                     cfg.W, cfg.GL, cfg.DG)
    E = src.shape[0]
    d = dloc >> 7
    dg = d // W
    w = d - dg * W
    m = src // CHS
    cell = ((dg * CH + m) * R + rel) * W + w
    NCELL = DG * CH * R * W
    kcell = core * NCELL + cell
    order = np.lexsort((src, kcell))
    src_s, core_s, m_s = src[order], core[order], m[order]
    dloc_s = dloc[order]
    kcell_s = kcell[order]

    cell_cnt = np.bincount(kcell, minlength=NC * NCELL)
    P = cell_cnt.reshape(NC, DG, CH, R, W).max(axis=0)
    P[:, 0, :, :] = np.maximum(P[:, 0, :, :], 16)
    for dd in range(DT, DG * W):
        P[dd // W, :, :, dd % W] = 0

    Ls = P.sum(axis=(2, 3))
    Lpad = ((Ls + 127) // 128) * 128
    cell_off = np.zeros((DG, CH, R, W), np.int64)
    for gg in range(DG):
        for mm in range(CH):
            acc = 0
            for r in range(R):
                for ww in range(W):
                    cell_off[gg, mm, r, ww] = acc
                    acc += P[gg, mm, r, ww]
    stream_base = np.zeros((DG, CH), np.int64)
    run = 0
    for gg in range(DG):
        for mm in range(CH):
            stream_base[gg, mm] = run
            run += Lpad[gg, mm]
    TOT = int(run)

    cell_start = np.zeros(NC * NCELL + 1, np.int64)
    cell_start[1:] = np.cumsum(cell_cnt)
    rank = np.arange(E, dtype=np.int64) - cell_start[kcell_s]
    d_s = dloc_s >> 7
    dg_s = d_s // W
    w_s = d_s - dg_s * W
    rel_s = (kcell_s % NCELL // W) % R
    slot = (stream_base[dg_s, m_s] + cell_off[dg_s, m_s, rel_s, w_s] + rank)

    seg_arr = np.full((NC, TOT), -1.0, np.float32)
    seg_arr[core_s, slot] = (dloc_s & 127).astype(np.float32)
    srcl_arr = np.zeros((NC, TOT), np.int64)
    srcl_arr[core_s, slot] = src_s - m_s * CHS

    gathers = []
    g_by_grp = [[] for _ in range(DG)]
    colbase = 0
    for gg in range(DG):
        for mm in range(CH):
            L = int(Lpad[gg, mm])
            o = 0
            while o < L:
                gl = min(GL, L - o)
                g_by_grp[gg].append(len(gathers))
                gathers.append((gg, mm, gl, colbase,
                                int(stream_base[gg, mm]) + o))
                colbase += gl // 16
                o += gl
    tot_cols = colbase

    idx_w = np.zeros((NC, 16, tot_cols), np.int16)
    for (gg, mm, gl, cb, sb) in gathers:
        blk = srcl_arr[:, sb:sb + gl].reshape(NC, gl // 16, 16)
        idx_w[:, :, cb:cb + gl // 16] = blk.transpose(0, 2, 1)
    idx_rep = np.tile(idx_w, (1, 8, 1))

    n_inc = np.zeros((DT, R), np.int64)
    tmp = []
    for go, (gg, mm, gl, cb, sb) in enumerate(gathers):
        for t in range(gl // 128):
            lo = (sb - int(stream_base[gg, mm])) + t * 128
            hi = lo + 128
            for r in range(R):
                for ww in range(W):
                    dd = gg * W + ww
                    if dd >= DT or P[gg, mm, r, ww] == 0:
                        continue
                    clo = int(cell_off[gg, mm, r, ww])
                    chi = clo + int(P[gg, mm, r, ww])
                    a, b = max(lo, clo), min(hi, chi)
                    if a >= b:
                        continue
                    tmp.append((go, t, dd, r, int(stream_base[gg, mm]), lo, a, b))
                    n_inc[dd, r] += 1
    seen = np.zeros((DT, R), np.int64)
    items = []
    iseg = np.full((NC, 128, len(tmp)), -1.0, np.float32)
    for it_i, (go, t, dd, r, sb0, lo, a, b) in enumerate(tmp):
        seen[dd, r] += 1
        items.append((go, t, dd, r, seen[dd, r] == 1,
                      seen[dd, r] == n_inc[dd, r]))
        iseg[:, a - lo:b - lo, it_i] = seg_arr[:, sb0 + a:sb0 + b]
    return dict(gathers=gathers, g_by_grp=g_by_grp, items=items,
                tot_cols=tot_cols, NIT=len(items), idx_rep=idx_rep, iseg=iseg)


def make_rect(cfg, recip_tab):
    NC, ND, DT, R, S3 = cfg.NC, cfg.ND, cfg.DT, cfg.R, cfg.S3
    rect = np.zeros((NC, DT * S3), np.float32)
    for c in range(NC):
        for dd in range(DT):
            base = c * ND + dd * 128
            nvalid = min(128, ND - dd * 128)
            for r in range(R):
                rect[c, dd * S3 + r * 128:dd * S3 + r * 128 + nvalid] = \
                    recip_tab[r, base:base + nvalid]
    return np.broadcast_to(rect[:, None, :], (NC, 128, DT * S3))


def build(cfg, s1, s2, debug=False):
    nc_ = bacc.Bacc("TRN2", target_bir_lowering=False, debug=False,
                    num_devices=cfg.NC, num_swdge_queues=4)
    N, F, H, EMB, R, B = cfg.N, cfg.F, cfg.H, cfg.EMB, cfg.R, cfg.B
    DT, CH, CHS, ND, NDP, S3, DG = (cfg.DT, cfg.CH, cfg.CHS, cfg.ND,
                                    cfg.NDP, cfg.S3, cfg.DG)
    TOT1, NIT1 = s1['TOT'], s1['NIT']
    NIT2, tot_cols = s2['NIT'], s2['tot_cols']
    it1_by_grp = [[] for _ in range(DG)]
    for it_i, it in enumerate(s1['items']):
        it1_by_grp[s1['chunks'][it[0]][0]].append(it_i)
    it2_by_grp = [[] for _ in range(DG)]
    for it_i, it in enumerate(s2['items']):
        it2_by_grp[s2['gathers'][it[0]][0]].append(it_i)

    t_xs = nc_.dram_tensor("xs", [TOT1, F], F16, kind="ExternalInput")
    t_xT = nc_.dram_tensor("xTc", [F, NDP], F16, kind="ExternalInput")
    t_idx = nc_.dram_tensor("idxw", [128, tot_cols], I16, kind="ExternalInput")
    t_iseg1 = nc_.dram_tensor("iseg1", [128, NIT1], F32, kind="ExternalInput")
    t_iseg2 = nc_.dram_tensor("iseg2", [128, NIT2], F32, kind="ExternalInput")
    t_rect = nc_.dram_tensor("rect", [128, DT * S3], F32, kind="ExternalInput")
    t_w1 = nc_.dram_tensor("w1", [F, R * H], F16, kind="ExternalInput")
    t_wr1 = nc_.dram_tensor("wr1", [F, H], F16, kind="ExternalInput")
    t_b1 = nc_.dram_tensor("b1", [1, H], F16, kind="ExternalInput")
    t_w2 = nc_.dram_tensor("w2", [H, R * EMB], F16, kind="ExternalInput")
    t_wr2 = nc_.dram_tensor("wr2", [H, EMB], F16, kind="ExternalInput")
    t_b2 = nc_.dram_tensor("b2", [1, EMB], F16, kind="ExternalInput")
    t_fca = nc_.dram_tensor("fca", [EMB, H], F16, kind="ExternalInput")
    t_fcb = nc_.dram_tensor("fcb", [EMB, H], F16, kind="ExternalInput")
    t_fbias = nc_.dram_tensor("fbias", [1, H], F16, kind="ExternalInput")
    t_nest = nc_.dram_tensor("nestw", [128, B // 16], I16, kind="ExternalInput")
    t_food = nc_.dram_tensor("foodw", [128, B // 16], I16, kind="ExternalInput")
    t_out = nc_.dram_tensor("out", [B, H], F32, kind="ExternalOutput")

    if debug:
        t_dbg_h1 = nc_.dram_tensor("dbg_h1", [NDP, H], F16,
                                   kind="ExternalOutput")
    h1_part = nc_.dram_tensor("h1_part", [NDP, H], F16, kind="Internal")
    h1_full = nc_.dram_tensor("h1_full", [N, H], F16, kind="Internal",
                              addr_space="Shared")
    nd_part = nc_.dram_tensor("nd_part", [NDP + 1, EMB], F32, kind="Internal")
    cc_fin = nc_.dram_tensor("cc_fin", [B, H], F16, kind="Internal")
    cc_fin_o = nc_.dram_tensor("cc_fin_o", [B, H], F16, kind="Internal",
                               addr_space="Shared")

    qctr = [0]

    with tile.TileContext(nc_) as tc:
        with tc.tile_pool(name="const", bufs=1) as cpool, \
             tc.tile_pool(name="big", bufs=1) as bigp, \
             tc.tile_pool(name="ms1", bufs=4) as ms1p, \
             tc.tile_pool(name="ms2", bufs=27) as ms2p, \
             tc.tile_pool(name="s", bufs=28) as sp, \
             tc.tile_pool(name="rec", bufs=6) as recp, \
             tc.tile_pool(name="ev", bufs=3) as evp, \
             tc.tile_pool(name="work", bufs=4) as wp, \
             tc.tile_pool(name="gath", bufs=1) as gp, \
             tc.tile_pool(name="pa", bufs=5, space="PSUM") as pap, \
             tc.tile_pool(name="pb", bufs=2, space="PSUM") as pbp, \
             tc.tile_pool(name="pc", bufs=1, space="PSUM") as pcp:

            c_i = cpool.tile([128, 128], I32)
            nc_.gpsimd.iota(c_i[:], pattern=[[1, 128]], base=0,
                            channel_multiplier=0)
            cw16 = cpool.tile([128, 128], F16)
            nc_.vector.tensor_copy(cw16[:], c_i[:])
            ones1 = cpool.tile([1, 128], F16)
            nc_.vector.memset(ones1[:], 1.0)
            ident = cpool.tile([128, 128], F16)
            make_identity(nc_, ident[:])

            idx_sb = bigp.tile([128, tot_cols], I16)
            for q in range(4):
                a, b = q * tot_cols // 4, (q + 1) * tot_cols // 4
                nc_.sync.dma_start(out=idx_sb[:, a:b], in_=t_idx[:, a:b])
            iseg1_sb = bigp.tile([128, NIT1], F32)
            nc_.sync.dma_start(out=iseg1_sb[:], in_=t_iseg1[:])
            iseg2_sb = bigp.tile([128, NIT2], F32)
            for q in range(4):
                a, b = q * NIT2 // 4, (q + 1) * NIT2 // 4
                nc_.sync.dma_start(out=iseg2_sb[:, a:b], in_=t_iseg2[:, a:b])
            xT_sb = bigp.tile([128, NDP], F16)
            nc_.sync.dma_start(out=xT_sb[:], in_=t_xT[:])
            h1T_sb = xT_sb

            w1_sb = cpool.tile([F, R * H], F16)
            nc_.sync.dma_start(out=w1_sb[:], in_=t_w1[:])
            wr1_sb = cpool.tile([F, H], F16)
            nc_.sync.dma_start(out=wr1_sb[:], in_=t_wr1[:])
            b1_sb = cpool.tile([1, H], F16)
            nc_.sync.dma_start(out=b1_sb[:], in_=t_b1[:])
            w2_sb = cpool.tile([H, R * EMB], F16)
            nc_.sync.dma_start(out=w2_sb[:], in_=t_w2[:])
            wr2_sb = cpool.tile([H, EMB], F16)
            nc_.sync.dma_start(out=wr2_sb[:], in_=t_wr2[:])
            b2_sb = cpool.tile([1, EMB], F16)
            nc_.sync.dma_start(out=b2_sb[:], in_=t_b2[:])
            fca_sb = cpool.tile([EMB, H], F16)
            nc_.sync.dma_start(out=fca_sb[:], in_=t_fca[:])
            fcb_sb = cpool.tile([EMB, H], F16)
            nc_.sync.dma_start(out=fcb_sb[:], in_=t_fcb[:])
            fbias_sb = cpool.tile([1, H], F16)
            nc_.sync.dma_start(out=fbias_sb[:], in_=t_fbias[:])
            nest_sb = cpool.tile([128, B // 16], I16)
            nc_.sync.dma_start(out=nest_sb[:], in_=t_nest[:])
            food_sb = cpool.tile([128, B // 16], I16)
            nc_.sync.dma_start(out=food_sb[:], in_=t_food[:])

            def tail(dd, psumA, wrel_sb, wroot_sb, bias_sb, HH, rootT_sb,
                     out_part, relu, make_h1T, dbg, odt=F16):
                rec = recp.tile([128, S3], F32, tag="rec")
                nc_.scalar.dma_start(out=rec[:],
                                     in_=t_rect[:, dd * S3:(dd + 1) * S3])
                ev = evp.tile([128, S3], F16, tag="ev")
                nc_.vector.tensor_tensor(out=ev[:], in0=psumA[:], in1=rec[:],
                                         op=mybir.AluOpType.mult)
                psumB = pbp.tile([128, HH], F32, tag="B", space="PSUM")
                for r in range(R):
                    nc_.tensor.matmul(out=psumB[:],
                                      lhsT=ev[:, r * 128:(r + 1) * 128],
                                      rhs=wrel_sb[:, r * HH:(r + 1) * HH],
                                      start=(r == 0), stop=False)
                nc_.tensor.matmul(out=psumB[:],
                                  lhsT=rootT_sb[:, dd * 128:(dd + 1) * 128],
                                  rhs=wroot_sb[:], start=False, stop=False)
                nc_.tensor.matmul(out=psumB[:], lhsT=ones1[:1, :],
                                  rhs=bias_sb[:1, :], start=False, stop=True)
                o_sb = wp.tile([128, HH], odt, tag="osb")
                if relu:
                    nc_.scalar.activation(
                        out=o_sb[:], in_=psumB[:],
                        func=mybir.ActivationFunctionType.Relu)
                else:
                    nc_.scalar.copy(out=o_sb[:], in_=psumB[:])
                if make_h1T:
                    nc_.gpsimd.dma_start(
                        out=out_part[dd * 128:(dd + 1) * 128, :], in_=o_sb[:])
                else:
                    nc_.sync.dma_start(
                        out=out_part[dd * 128:(dd + 1) * 128, :], in_=o_sb[:])
                if dbg:
                    nc_.sync.dma_start(
                        out=t_dbg_h1[dd * 128:(dd + 1) * 128, :], in_=o_sb[:])
                if make_h1T:
                    psumC = pcp.tile([128, 128], F16, tag="C", space="PSUM")
                    nc_.tensor.transpose(out=psumC[:], in_=o_sb[:],
                                         identity=ident[:])
                    nc_.scalar.copy(out=h1T_sb[:, dd * 128:(dd + 1) * 128],
                                    in_=psumC[:])

            def emit_items(msgs, items, iseg_sb, it_list, psumA, act_nth):
                for k, it_i in enumerate(it_list):
                    go, t, dd, rr, first, last = items[it_i]
                    if dd not in psumA:
                        pa_tile = pap.tile([128, S3], F32, tag="A",
                                           space="PSUM")
                        psumA[dd] = pa_tile
                    if k % act_nth == act_nth - 1:
                        # offload ~25% of one-hot generation to Activation:
                        # u = |seg - c|; S = relu(1 - u)  (exact for ints)
                        u = sp.tile([128, 128], F16, tag="U")
                        nc_.scalar.activation(
                            out=u[:], in_=cw16[:],
                            func=mybir.ActivationFunctionType.Abs,
                            bias=iseg_sb[:, it_i:it_i + 1], scale=-1.0)
                        S = sp.tile([128, 128], F16, tag="SA")
                        nc_.scalar.activation(
                            out=S[:], in_=u[:],
                            func=mybir.ActivationFunctionType.Relu,
                            bias=1.0, scale=-1.0)
                    else:
                        S = sp.tile([128, 128], F16, tag="S")
                        nc_.vector.tensor_scalar(
                            out=S[:], in0=cw16[:],
                            scalar1=iseg_sb[:, it_i:it_i + 1], scalar2=None,
                            op0=mybir.AluOpType.is_equal)
                    nc_.tensor.matmul(
                        out=psumA[dd][:, rr * 128:(rr + 1) * 128],
                        lhsT=msgs[go][:, t * F:(t + 1) * F],
                        rhs=S[:], start=bool(first), stop=bool(last))

            # ---- layer 1: sequential stream ----
            items1 = s1['items']
            for gg in range(DG):
                msgs = {}
                for co in s1['c_by_grp'][gg]:
                    _, cl, sb = s1['chunks'][co]
                    msg = ms1p.tile([128, (cl // 128) * F], F16, tag="m1")
                    nc_.sync.dma_start(
                        out=msg[:].rearrange("p (c e) -> p c e", e=F),
                        in_=t_xs[sb:sb + cl, :].rearrange(
                            "(c p) e -> p c e", p=128))
                    msgs[co] = msg
                psumA = {}
                emit_items(msgs, items1,
                           iseg1_sb,
                           sorted(it1_by_grp[gg],
                                  key=lambda i: (items1[i][3], i)),
                           psumA, 6)
                for dd in sorted(psumA.keys()):
                    tail(dd, psumA[dd], w1_sb, wr1_sb, b1_sb, H, xT_sb,
                         h1_part, relu=True, make_h1T=True, dbg=debug)

            nc_.gpsimd.collective_compute(
                "AllGather", mybir.AluOpType.bypass,
                replica_groups=[list(range(cfg.NC))],
                ins=[h1_part[:ND, :]], outs=[h1_full[:]])

            # ---- layer 2: SWDGE gathers from h1_full ----
            items2 = s2['items']
            gathers = s2['gathers']
            for gg in range(DG):
                msgs = {}
                for go in s2['g_by_grp'][gg]:
                    _, mm, gl, cb, _sb = gathers[go]
                    msg = ms2p.tile([128, (gl // 128) * F], F16, tag="m2")
                    nc_.gpsimd.dma_gather(
                        out_ap=msg[:].rearrange("p (c e) -> p c e", e=F),
                        in_ap=h1_full[mm * CHS:(mm + 1) * CHS, :],
                        idxs_ap=idx_sb[:, cb:cb + gl // 16],
                        num_idxs=gl, num_idxs_reg=gl, elem_size=F,
                        queue_num=qctr[0] % 4)
                    qctr[0] += 1
                    msgs[go] = msg
                psumA = {}
                emit_items(msgs, items2,
                           iseg2_sb,
                           sorted(it2_by_grp[gg],
                                  key=lambda i: (items2[i][3], i)),
                           psumA, 4)
                for dd in sorted(psumA.keys()):
                    tail(dd, psumA[dd], w2_sb, wr2_sb, b2_sb, EMB, h1T_sb,
                         nd_part, relu=False, make_h1T=False, dbg=False,
                         odt=F32)

            # ---- final pair MLP ----
            zrow = wp.tile([1, EMB], F32, tag="zrow")
            nc_.vector.memset(zrow[:], 0.0)
            nc_.sync.dma_start(out=nd_part[NDP:NDP + 1, :], in_=zrow[:1, :])
            nfall = gp.tile([128, (B // 128) * EMB], F32, tag="nfall")
            nc_.gpsimd.dma_gather(
                out_ap=nfall[:].rearrange("p (c e) -> p c e", e=EMB),
                in_ap=nd_part[:, :], idxs_ap=nest_sb[:, :B // 16],
                num_idxs=B, num_idxs_reg=B, elem_size=EMB, queue_num=0)
            ffall = gp.tile([128, (B // 128) * EMB], F32, tag="ffall")
            nc_.gpsimd.dma_gather(
                out_ap=ffall[:].rearrange("p (c e) -> p c e", e=EMB),
                in_ap=nd_part[:, :], idxs_ap=food_sb[:, :B // 16],
                num_idxs=B, num_idxs_reg=B, elem_size=EMB, queue_num=1)
            for pt in range(B // 128):
                nf = wp.tile([128, EMB], F16, tag="nf")
                nc_.vector.tensor_copy(nf[:], nfall[:, pt * EMB:(pt + 1) * EMB])
                ff = wp.tile([128, EMB], F16, tag="ff")
                nc_.vector.tensor_copy(ff[:], ffall[:, pt * EMB:(pt + 1) * EMB])
                psumC = pcp.tile([128, 128], F16, tag="C", space="PSUM")
                nc_.tensor.transpose(out=psumC[:EMB, :], in_=nf[:],
                                     identity=ident[:])
                nfT = wp.tile([EMB, 128], F16, tag="nfT")
                nc_.vector.tensor_copy(nfT[:], psumC[:EMB, :])
                psumC2 = pcp.tile([128, 128], F16, tag="C", space="PSUM")
                nc_.tensor.transpose(out=psumC2[:EMB, :], in_=ff[:],
                                     identity=ident[:])
                ffT = wp.tile([EMB, 128], F16, tag="ffT")
                nc_.vector.tensor_copy(ffT[:], psumC2[:EMB, :])
                psumD = pbp.tile([128, H], F32, tag="B", space="PSUM")
                nc_.tensor.matmul(out=psumD[:], lhsT=nfT[:], rhs=fca_sb[:],
                                  start=True, stop=False)
                nc_.tensor.matmul(out=psumD[:], lhsT=ffT[:], rhs=fcb_sb[:],
                                  start=False, stop=False)
                nc_.tensor.matmul(out=psumD[:], lhsT=ones1[:1, :],
                                  rhs=fbias_sb[:1, :], start=False, stop=True)
                fo = wp.tile([128, H], F16, tag="fo")
                nc_.vector.tensor_copy(fo[:], psumD[:])
                nc_.sync.dma_start(out=cc_fin[pt * 128:(pt + 1) * 128, :],
                                   in_=fo[:])
            nc_.gpsimd.collective_compute(
                "AllReduce", mybir.AluOpType.add,
                replica_groups=[list(range(cfg.NC))],
                ins=[cc_fin[:]], outs=[cc_fin_o[:]])
            for pt in range(B // 128):
                ft = wp.tile([128, H], F16, tag="ft")
                nc_.sync.dma_start(out=ft[:],
                                   in_=cc_fin_o[pt * 128:(pt + 1) * 128, :])
                fo2 = wp.tile([128, H], F32, tag="fo2")
                nc_.scalar.activation(out=fo2[:], in_=ft[:],
                                      func=mybir.ActivationFunctionType.Tanh)
                nc_.sync.dma_start(out=t_out[pt * 128:(pt + 1) * 128, :],
                                   in_=fo2[:])
    nc_.compile()
    return nc_


def kernel(x, edge_src, edge_dst, edge_type, edge_attr, nest, food,
           W_rel1, W_root1, b1, W_rel2, W_root2, b2, fc_W, fc_b,
           _cfg=FULL, _runner=None, _debug=False):
    cfg = _cfg
    x = np.asarray(x, np.float32)
    src, dst, rel, core, dloc, recip_tab = _common(
        cfg, np.asarray(edge_src), np.asarray(edge_dst), np.asarray(edge_type))
    s1 = preprocess_l1(cfg, src, rel, core, dloc)
    s2 = preprocess_l2(cfg, src, rel, core, dloc)
    rect_rep = make_rect(cfg, recip_tab)
    nc_ = build(cfg, s1, s2, debug=_debug)

    x16 = x.astype(np.float16)
    W_rel1 = np.asarray(W_rel1, np.float32)
    W_rel2 = np.asarray(W_rel2, np.float32)
    w1 = W_rel1.transpose(1, 0, 2).reshape(cfg.F, cfg.R * cfg.H).astype(np.float16)
    w2 = W_rel2.transpose(1, 0, 2).reshape(cfg.H, cfg.R * cfg.EMB).astype(np.float16)
    fc_W = np.asarray(fc_W, np.float32)
    nest = np.asarray(nest, np.int64)
    food = np.asarray(food, np.int64)

    xT = x16.T
    in_maps = []
    for c in range(cfg.NC):
        # host-side layer-1 gather into stream order; pad slots point at
        # row 0 (their seg=-1 one-hot column masks them to zero anyway)
        xs = x16[s1['srcrow'][c]]
        xTc = np.zeros((cfg.F, cfg.NDP), np.float16)
        xTc[:, :cfg.ND] = xT[:, c * cfg.ND:(c + 1) * cfg.ND]
        nl = nest - c * cfg.ND
        nest_c = np.where((nl >= 0) & (nl < cfg.ND), nl, cfg.NDP).astype(np.int32)
        fl = food - c * cfg.ND
        food_c = np.where((fl >= 0) & (fl < cfg.ND), fl, cfg.NDP).astype(np.int32)
        in_maps.append(dict(
            xs=xs, xTc=xTc, idxw=s2['idx_rep'][c],
            iseg1=s1['iseg'][c], iseg2=s2['iseg'][c],
            rect=np.ascontiguousarray(rect_rep[c]),
            w1=w1, wr1=np.asarray(W_root1, np.float16),
            b1=np.asarray(b1, np.float16).reshape(1, -1),
            w2=w2, wr2=np.asarray(W_root2, np.float16),
            b2=np.asarray(b2, np.float16).reshape(1, -1),
            fca=fc_W[:cfg.EMB].astype(np.float16),
            fcb=fc_W[cfg.EMB:].astype(np.float16),
            fbias=(np.asarray(fc_b, np.float32).reshape(1, -1) / cfg.NC
                   ).astype(np.float16),
            nestw=np.tile(nest_c.astype(np.int16).reshape(-1, 16).T, (8, 1)),
            foodw=np.tile(food_c.astype(np.int16).reshape(-1, 16).T, (8, 1)),
        ))
    runner = _runner or (lambda n, im: run_bass_kernel_spmd(
        n, im, core_ids=list(range(cfg.NC))))
    res = runner(nc_, in_maps)
    return res.results[0]["out"]


# revision 42
# speedup vs baseline: 1.0543x; 1.0543x over previous
"""RGCN 2-layer + pair-MLP Trainium2 kernel (8 NeuronCores, SPMD).

v3: fp16 hot path, edges sharded by dst range (12500 nodes/core).

Layer 1 needs no on-device gather at all: the host pre-sorts x rows into
edge-stream order (x_stream = x16[src, :] in (dst-tile-group, relation,
dst-tile) order), so the device just streams it sequentially with plain
DMAs. Layer 2 gathers h1 rows per edge with SWDGE dma_gather (1024-idx
gathers round-robined over 4 SWDGE queues; int16 indices force 4 source
chunks). Both layers segment-sum into PSUM via narrow [128,128] one-hot
matmuls (fp16, relation-major emission so each PSUM bank has only one
open accumulation group), apply the mean reciprocal at PSUM eviction
(DVE tensor_tensor against a per-dst-tile recip tile), then run the
per-relation transforms + root + bias as fp16 matmuls. h1 moves between
cores as a fp16 AllGather; the final pair MLP uses the local-partial +
AllReduce trick.
"""
import sys
sys.path.insert(0, '/opt/trn_rl_repo')

import numpy as np
import concourse.bass as bass
import concourse.bacc as bacc
import concourse.tile as tile
import concourse.mybir as mybir
from concourse.bass_utils import run_bass_kernel_spmd
from concourse.masks import make_identity

F32 = mybir.dt.float32
F16 = mybir.dt.float16
I32 = mybir.dt.int32
I16 = mybir.dt.int16


class Cfg:
    def __init__(self, N, F, H, EMB, R, E, B, NC, CH=4, W=3, GL=1024, SC=4096):
        self.N, self.F, self.H, self.EMB, self.R, self.E, self.B = N, F, H, EMB, R, E, B
        self.NC = NC
        self.CH = CH                      # layer-2 src chunks (int16 idx)
        self.W = W                        # dst tiles per group
        self.GL = GL                      # idxs per dma_gather (layer 2)
        self.SC = SC                      # rows per stream DMA (layer 1)
        self.ND = N // NC
        self.DT = (self.ND + 127) // 128
        self.NDP = self.DT * 128
        self.CHS = N // CH
        self.DG = (self.DT + W - 1) // W
        self.S3 = self.R * 128


FULL = Cfg(N=100000, F=128, H=128, EMB=64, R=3, E=1600000, B=1024, NC=8)


def _common(cfg, edge_src, edge_dst, edge_type):
    N, NC, ND, R = cfg.N, cfg.NC, cfg.ND, cfg.R
    src = edge_src.astype(np.int64)
    dst = edge_dst.astype(np.int64)
    rel = edge_type.astype(np.int64)
    cnt = np.bincount(rel * N + dst, minlength=R * N).astype(np.float32)
    recip_tab = (1.0 / np.maximum(cnt, 1.0)).reshape(R, N)
    core = dst // ND
    dloc = dst - core * ND
    return src, dst, rel, core, dloc, recip_tab


def preprocess_l1(cfg, src, rel, core, dloc):
    """Layer 1: single host-sorted stream per core, cells (dg, r, w)."""
    NC, DT, R, W, DG, SC = cfg.NC, cfg.DT, cfg.R, cfg.W, cfg.DG, cfg.SC
    E = src.shape[0]
    d = dloc >> 7
    dg = d // W
    w = d - dg * W
    cell = (dg * R + rel) * W + w
    NCELL = DG * R * W
    kcell = core * NCELL + cell
    order = np.lexsort((src, kcell))
    src_s, core_s = src[order], core[order]
    dloc_s = dloc[order]
    kcell_s = kcell[order]

    cell_cnt = np.bincount(kcell, minlength=NC * NCELL)
    P = cell_cnt.reshape(NC, DG, R, W).max(axis=0)
    P[:, :, :] = np.maximum(P, 1)          # keep every (d, r) group alive
    for dd in range(DT, DG * W):
        P[dd // W, :, dd % W] = 0

    cell_off = np.zeros((DG, R, W), np.int64)
    stream_base = np.zeros(DG, np.int64)
    run = 0
    for gg in range(DG):
        stream_base[gg] = run
        acc = 0
        for r in range(R):
            for ww in range(W):
                cell_off[gg, r, ww] = acc
                acc += P[gg, r, ww]
        run += ((acc + 127) // 128) * 128
    TOT = int(run)
    grp_len = np.zeros(DG, np.int64)
    for gg in range(DG):
        nxt = stream_base[gg + 1] if gg + 1 < DG else TOT
        grp_len[gg] = nxt - stream_base[gg]

    cell_start = np.zeros(NC * NCELL + 1, np.int64)
    cell_start[1:] = np.cumsum(cell_cnt)
    rank = np.arange(E, dtype=np.int64) - cell_start[kcell_s]
    d_s = dloc_s >> 7
    dg_s = d_s // W
    w_s = d_s - dg_s * W
    rel_s2 = (kcell_s % NCELL // W) % R
    slot = stream_base[dg_s] + cell_off[dg_s, rel_s2, w_s] + rank

    seg_arr = np.full((NC, TOT), -1.0, np.float32)
    seg_arr[core_s, slot] = (dloc_s & 127).astype(np.float32)
    srcrow = np.zeros((NC, TOT), np.int64)
    srcrow[core_s, slot] = src_s

    # chunks: per group, sequential DMAs of <= SC rows (multiples of 128)
    chunks = []                      # (gg, clen, sbase)
    c_by_grp = [[] for _ in range(DG)]
    for gg in range(DG):
        L = int(grp_len[gg])
        o = 0
        while o < L:
            cl = min(SC, L - o)
            c_by_grp[gg].append(len(chunks))
            chunks.append((gg, cl, int(stream_base[gg]) + o))
            o += cl

    # items
    n_inc = np.zeros((DT, R), np.int64)
    tmp = []
    for co, (gg, cl, sb) in enumerate(chunks):
        for t in range(cl // 128):
            lo = (sb - int(stream_base[gg])) + t * 128
            hi = lo + 128
            for r in range(R):
                for ww in range(W):
                    dd = gg * W + ww
                    if dd >= DT or P[gg, r, ww] == 0:
                        continue
                    clo = int(cell_off[gg, r, ww])
                    chi = clo + int(P[gg, r, ww])
                    a, b = max(lo, clo), min(hi, chi)
                    if a >= b:
                        continue
                    tmp.append((co, t, dd, r, int(stream_base[gg]), lo, a, b))
                    n_inc[dd, r] += 1
    seen = np.zeros((DT, R), np.int64)
    items = []
    iseg = np.full((NC, 128, len(tmp)), -1.0, np.float32)
    for it_i, (co, t, dd, r, sb0, lo, a, b) in enumerate(tmp):
        seen[dd, r] += 1
        items.append((co, t, dd, r, seen[dd, r] == 1,
                      seen[dd, r] == n_inc[dd, r]))
        iseg[:, a - lo:b - lo, it_i] = seg_arr[:, sb0 + a:sb0 + b]
    return dict(chunks=chunks, c_by_grp=c_by_grp, items=items, TOT=TOT,
                NIT=len(items), srcrow=srcrow, iseg=iseg)


def preprocess_l2(cfg, src, rel, core, dloc):
    """Layer 2: SWDGE gathers, cells (dg, m, r, w), int16 chunked idx."""
    NC, DT, CH, CHS, R, W, GL, DG = (cfg.NC, cfg.DT, cfg.CH, cfg.CHS, cfg.R,
                                     cfg.W, cfg.GL, cfg.DG)
    E = src.shape[0]
    d = dloc >> 7
    dg = d // W
    w = d - dg * W
    m = src // CHS
    cell = ((dg * CH + m) * R + rel) * W + w
    NCELL = DG * CH * R * W
    kcell = core * NCELL + cell
    order = np.lexsort((src, kcell))
    src_s, core_s, m_s = src[order], core[order], m[order]
    dloc_s = dloc[order]
    kcell_s = kcell[order]

    cell_cnt = np.bincount(kcell, minlength=NC * NCELL)
    P = cell_cnt.reshape(NC, DG, CH, R, W).max(axis=0)
    P[:, 0, :, :] = np.maximum(P[:, 0, :, :], 16)
    for dd in range(DT, DG * W):
        P[dd // W, :, :, dd % W] = 0

    Ls = P.sum(axis=(2, 3))
    Lpad = ((Ls + 127) // 128) * 128
    cell_off = np.zeros((DG, CH, R, W), np.int64)
    for gg in range(DG):
        for mm in range(CH):
            acc = 0
            for r in range(R):
                for ww in range(W):
                    cell_off[gg, mm, r, ww] = acc
                    acc += P[gg, mm, r, ww]
    stream_base = np.zeros((DG, CH), np.int64)
    run = 0
    for gg in range(DG):
        for mm in range(CH):
            stream_base[gg, mm] = run
            run += Lpad[gg, mm]
    TOT = int(run)

    cell_start = np.zeros(NC * NCELL + 1, np.int64)
    cell_start[1:] = np.cumsum(cell_cnt)
    rank = np.arange(E, dtype=np.int64) - cell_start[kcell_s]
    d_s = dloc_s >> 7
    dg_s = d_s // W
    w_s = d_s - dg_s * W
    rel_s = (kcell_s % NCELL // W) % R
    slot = (stream_base[dg_s, m_s] + cell_off[dg_s, m_s, rel_s, w_s] + rank)

    seg_arr = np.full((NC, TOT), -1.0, np.float32)
    seg_arr[core_s, slot] = (dloc_s & 127).astype(np.float32)
    srcl_arr = np.zeros((NC, TOT), np.int64)
    srcl_arr[core_s, slot] = src_s - m_s * CHS

    gathers = []
    g_by_grp = [[] for _ in range(DG)]
    colbase = 0
    for gg in range(DG):
        for mm in range(CH):
            L = int(Lpad[gg, mm])
            o = 0
            while o < L:
                gl = min(GL, L - o)
                g_by_grp[gg].append(len(gathers))
                gathers.append((gg, mm, gl, colbase,
                                int(stream_base[gg, mm]) + o))
                colbase += gl // 16
                o += gl
    tot_cols = colbase

    idx_w = np.zeros((NC, 16, tot_cols), np.int16)
    for (gg, mm, gl, cb, sb) in gathers:
        blk = srcl_arr[:, sb:sb + gl].reshape(NC, gl // 16, 16)
        idx_w[:, :, cb:cb + gl // 16] = blk.transpose(0, 2, 1)
    idx_rep = np.tile(idx_w, (1, 8, 1))

    n_inc = np.zeros((DT, R), np.int64)
    tmp = []
    for go, (gg, mm, gl, cb, sb) in enumerate(gathers):
        for t in range(gl // 128):
            lo = (sb - int(stream_base[gg, mm])) + t * 128
            hi = lo + 128
            for r in range(R):
                for ww in range(W):
                    dd = gg * W + ww
                    if dd >= DT or P[gg, mm, r, ww] == 0:
                        continue
                    clo = int(cell_off[gg, mm, r, ww])
                    chi = clo + int(P[gg, mm, r, ww])
                    a, b = max(lo, clo), min(hi, chi)
                    if a >= b:
                        continue
                    tmp.append((go, t, dd, r, int(stream_base[gg, mm]), lo, a, b))
                    n_inc[dd, r] += 1
    seen = np.zeros((DT, R), np.int64)
    items = []
    iseg = np.full((NC, 128, len(tmp)), -1.0, np.float32)
    for it_i, (go, t, dd, r, sb0, lo, a, b) in enumerate(tmp):
        seen[dd, r] += 1
        items.append((go, t, dd, r, seen[dd, r] == 1,
                      seen[dd, r] == n_inc[dd, r]))
        iseg[:, a - lo:b - lo, it_i] = seg_arr[:, sb0 + a:sb0 + b]
    return dict(gathers=gathers, g_by_grp=g_by_grp, items=items,
                tot_cols=tot_cols, NIT=len(items), idx_rep=idx_rep, iseg=iseg)


def make_rect(cfg, recip_tab):
    NC, ND, DT, R, S3 = cfg.NC, cfg.ND, cfg.DT, cfg.R, cfg.S3
    rect = np.zeros((NC, DT * S3), np.float32)
    for c in range(NC):
        for dd in range(DT):
            base = c * ND + dd * 128
            nvalid = min(128, ND - dd * 128)
            for r in range(R):
                rect[c, dd * S3 + r * 128:dd * S3 + r * 128 + nvalid] = \
                    recip_tab[r, base:base + nvalid]
    return np.broadcast_to(rect[:, None, :], (NC, 128, DT * S3))


def build(cfg, s1, s2, debug=False):
    nc_ = bacc.Bacc("TRN2", target_bir_lowering=False, debug=False,
                    num_devices=cfg.NC, num_swdge_queues=4)
    N, F, H, EMB, R, B = cfg.N, cfg.F, cfg.H, cfg.EMB, cfg.R, cfg.B
    DT, CH, CHS, ND, NDP, S3, DG = (cfg.DT, cfg.CH, cfg.CHS, cfg.ND,
                                    cfg.NDP, cfg.S3, cfg.DG)
    TOT1, NIT1 = s1['TOT'], s1['NIT']
    NIT2, tot_cols = s2['NIT'], s2['tot_cols']
    it1_by_grp = [[] for _ in range(DG)]
    for it_i, it in enumerate(s1['items']):
        it1_by_grp[s1['chunks'][it[0]][0]].append(it_i)
    it2_by_grp = [[] for _ in range(DG)]
    for it_i, it in enumerate(s2['items']):
        it2_by_grp[s2['gathers'][it[0]][0]].append(it_i)

    t_xs = nc_.dram_tensor("xs", [TOT1, F], F16, kind="ExternalInput")
    t_xT = nc_.dram_tensor("xTc", [F, NDP], F16, kind="ExternalInput")
    t_idx = nc_.dram_tensor("idxw", [128, tot_cols], I16, kind="ExternalInput")
    t_iseg1 = nc_.dram_tensor("iseg1", [128, NIT1], F32, kind="ExternalInput")
    t_iseg2 = nc_.dram_tensor("iseg2", [128, NIT2], F32, kind="ExternalInput")
    t_rect = nc_.dram_tensor("rect", [128, DT * S3], F32, kind="ExternalInput")
    t_w1 = nc_.dram_tensor("w1", [F, R * H], F16, kind="ExternalInput")
    t_wr1 = nc_.dram_tensor("wr1", [F, H], F16, kind="ExternalInput")
    t_b1 = nc_.dram_tensor("b1", [1, H], F16, kind="ExternalInput")
    t_w2 = nc_.dram_tensor("w2", [H, R * EMB], F16, kind="ExternalInput")
    t_wr2 = nc_.dram_tensor("wr2", [H, EMB], F16, kind="ExternalInput")
    t_b2 = nc_.dram_tensor("b2", [1, EMB], F16, kind="ExternalInput")
    t_fca = nc_.dram_tensor("fca", [EMB, H], F16, kind="ExternalInput")
    t_fcb = nc_.dram_tensor("fcb", [EMB, H], F16, kind="ExternalInput")
    t_fbias = nc_.dram_tensor("fbias", [1, H], F16, kind="ExternalInput")
    t_nest = nc_.dram_tensor("nestw", [128, B // 16], I16, kind="ExternalInput")
    t_food = nc_.dram_tensor("foodw", [128, B // 16], I16, kind="ExternalInput")
    t_out = nc_.dram_tensor("out", [B, H], F32, kind="ExternalOutput")

    if debug:
        t_dbg_h1 = nc_.dram_tensor("dbg_h1", [NDP, H], F16,
                                   kind="ExternalOutput")
    h1_part = nc_.dram_tensor("h1_part", [NDP, H], F16, kind="Internal")
    h1_full = nc_.dram_tensor("h1_full", [N, H], F16, kind="Internal",
                              addr_space="Shared")
    nd_part = nc_.dram_tensor("nd_part", [NDP + 1, EMB], F32, kind="Internal")
    cc_fin = nc_.dram_tensor("cc_fin", [B, H], F32, kind="Internal")
    cc_fin_o = nc_.dram_tensor("cc_fin_o", [B, H], F32, kind="Internal",
                               addr_space="Shared")

    qctr = [0]

    with tile.TileContext(nc_) as tc:
        with tc.tile_pool(name="const", bufs=1) as cpool, \
             tc.tile_pool(name="big", bufs=1) as bigp, \
             tc.tile_pool(name="ms1", bufs=4) as ms1p, \
             tc.tile_pool(name="ms2", bufs=26) as ms2p, \
             tc.tile_pool(name="s", bufs=24) as sp, \
             tc.tile_pool(name="rec", bufs=6) as recp, \
             tc.tile_pool(name="ev", bufs=3) as evp, \
             tc.tile_pool(name="work", bufs=4) as wp, \
             tc.tile_pool(name="gath", bufs=1) as gp, \
             tc.tile_pool(name="pa", bufs=5, space="PSUM") as pap, \
             tc.tile_pool(name="pb", bufs=2, space="PSUM") as pbp, \
             tc.tile_pool(name="pc", bufs=1, space="PSUM") as pcp:

            c_i = cpool.tile([128, 128], I32)
            nc_.gpsimd.iota(c_i[:], pattern=[[1, 128]], base=0,
                            channel_multiplier=0)
            cw16 = cpool.tile([128, 128], F16)
            nc_.vector.tensor_copy(cw16[:], c_i[:])
            ones1 = cpool.tile([1, 128], F16)
            nc_.vector.memset(ones1[:], 1.0)
            ident = cpool.tile([128, 128], F16)
            make_identity(nc_, ident[:])

            idx_sb = bigp.tile([128, tot_cols], I16)
            for q in range(4):
                a, b = q * tot_cols // 4, (q + 1) * tot_cols // 4
                nc_.sync.dma_start(out=idx_sb[:, a:b], in_=t_idx[:, a:b])
            iseg1_sb = bigp.tile([128, NIT1], F32)
            nc_.sync.dma_start(out=iseg1_sb[:], in_=t_iseg1[:])
            iseg2_sb = bigp.tile([128, NIT2], F32)
            for q in range(4):
                a, b = q * NIT2 // 4, (q + 1) * NIT2 // 4
                nc_.sync.dma_start(out=iseg2_sb[:, a:b], in_=t_iseg2[:, a:b])
            xT_sb = bigp.tile([128, NDP], F16)
            nc_.sync.dma_start(out=xT_sb[:], in_=t_xT[:])
            h1T_sb = xT_sb

            w1_sb = cpool.tile([F, R * H], F16)
            nc_.sync.dma_start(out=w1_sb[:], in_=t_w1[:])
            wr1_sb = cpool.tile([F, H], F16)
            nc_.sync.dma_start(out=wr1_sb[:], in_=t_wr1[:])
            b1_sb = cpool.tile([1, H], F16)
            nc_.sync.dma_start(out=b1_sb[:], in_=t_b1[:])
            w2_sb = cpool.tile([H, R * EMB], F16)
            nc_.sync.dma_start(out=w2_sb[:], in_=t_w2[:])
            wr2_sb = cpool.tile([H, EMB], F16)
            nc_.sync.dma_start(out=wr2_sb[:], in_=t_wr2[:])
            b2_sb = cpool.tile([1, EMB], F16)
            nc_.sync.dma_start(out=b2_sb[:], in_=t_b2[:])
            fca_sb = cpool.tile([EMB, H], F16)
            nc_.sync.dma_start(out=fca_sb[:], in_=t_fca[:])
            fcb_sb = cpool.tile([EMB, H], F16)
            nc_.sync.dma_start(out=fcb_sb[:], in_=t_fcb[:])
            fbias_sb = cpool.tile([1, H], F16)
            nc_.sync.dma_start(out=fbias_sb[:], in_=t_fbias[:])
            nest_sb = cpool.tile([128, B // 16], I16)
            nc_.sync.dma_start(out=nest_sb[:], in_=t_nest[:])
            food_sb = cpool.tile([128, B // 16], I16)
            nc_.sync.dma_start(out=food_sb[:], in_=t_food[:])

            def tail(dd, psumA, wrel_sb, wroot_sb, bias_sb, HH, rootT_sb,
                     out_part, relu, make_h1T, dbg, odt=F16):
                rec = recp.tile([128, S3], F32, tag="rec")
                nc_.scalar.dma_start(out=rec[:],
                                     in_=t_rect[:, dd * S3:(dd + 1) * S3])
                ev = evp.tile([128, S3], F16, tag="ev")
                nc_.vector.tensor_tensor(out=ev[:], in0=psumA[:], in1=rec[:],
                                         op=mybir.AluOpType.mult)
                psumB = pbp.tile([128, HH], F32, tag="B", space="PSUM")
                for r in range(R):
                    nc_.tensor.matmul(out=psumB[:],
                                      lhsT=ev[:, r * 128:(r + 1) * 128],
                                      rhs=wrel_sb[:, r * HH:(r + 1) * HH],
                                      start=(r == 0), stop=False)
                nc_.tensor.matmul(out=psumB[:],
                                  lhsT=rootT_sb[:, dd * 128:(dd + 1) * 128],
                                  rhs=wroot_sb[:], start=False, stop=False)
                nc_.tensor.matmul(out=psumB[:], lhsT=ones1[:1, :],
                                  rhs=bias_sb[:1, :], start=False, stop=True)
                o_sb = wp.tile([128, HH], odt, tag="osb")
                if relu:
                    nc_.scalar.activation(
                        out=o_sb[:], in_=psumB[:],
                        func=mybir.ActivationFunctionType.Relu)
                else:
                    nc_.scalar.copy(out=o_sb[:], in_=psumB[:])
                if make_h1T:
                    nc_.gpsimd.dma_start(
                        out=out_part[dd * 128:(dd + 1) * 128, :], in_=o_sb[:])
                else:
                    nc_.sync.dma_start(
                        out=out_part[dd * 128:(dd + 1) * 128, :], in_=o_sb[:])
                if dbg:
                    nc_.sync.dma_start(
                        out=t_dbg_h1[dd * 128:(dd + 1) * 128, :], in_=o_sb[:])
                if make_h1T:
                    psumC = pcp.tile([128, 128], F16, tag="C", space="PSUM")
                    nc_.tensor.transpose(out=psumC[:], in_=o_sb[:],
                                         identity=ident[:])
                    nc_.scalar.copy(out=h1T_sb[:, dd * 128:(dd + 1) * 128],
                                    in_=psumC[:])

            def emit_items(msgs, items, iseg_sb, it_list, psumA, act_nth):
                for k, it_i in enumerate(it_list):
                    go, t, dd, rr, first, last = items[it_i]
                    if dd not in psumA:
                        pa_tile = pap.tile([128, S3], F32, tag="A",
                                           space="PSUM")
                        psumA[dd] = pa_tile
                    if k % act_nth == act_nth - 1:
                        # offload ~25% of one-hot generation to Activation:
                        # u = |seg - c|; S = relu(1 - u)  (exact for ints)
                        u = sp.tile([128, 128], F16, tag="U")
                        nc_.scalar.activation(
                            out=u[:], in_=cw16[:],
                            func=mybir.ActivationFunctionType.Abs,
                            bias=iseg_sb[:, it_i:it_i + 1], scale=-1.0)
                        S = sp.tile([128, 128], F16, tag="SA")
                        nc_.scalar.activation(
                            out=S[:], in_=u[:],
                            func=mybir.ActivationFunctionType.Relu,
                            bias=1.0, scale=-1.0)
                    else:
                        S = sp.tile([128, 128], F16, tag="S")
                        nc_.vector.tensor_scalar(
                            out=S[:], in0=cw16[:],
                            scalar1=iseg_sb[:, it_i:it_i + 1], scalar2=None,
                            op0=mybir.AluOpType.is_equal)
                    nc_.tensor.matmul(
                        out=psumA[dd][:, rr * 128:(rr + 1) * 128],
                        lhsT=msgs[go][:, t * F:(t + 1) * F],
                        rhs=S[:], start=bool(first), stop=bool(last))

            # ---- layer 1: sequential stream ----
            items1 = s1['items']
            for gg in range(DG):
                msgs = {}
                for co in s1['c_by_grp'][gg]:
                    _, cl, sb = s1['chunks'][co]
                    msg = ms1p.tile([128, (cl // 128) * F], F16, tag="m1")
                    eng = nc_.sync if co % 2 == 0 else nc_.gpsimd
                    eng.dma_start(
                        out=msg[:].rearrange("p (c e) -> p c e", e=F),
                        in_=t_xs[sb:sb + cl, :].rearrange(
                            "(c p) e -> p c e", p=128))
                    msgs[co] = msg
                psumA = {}
                emit_items(msgs, items1,
                           iseg1_sb,
                           sorted(it1_by_grp[gg],
                                  key=lambda i: (items1[i][3], i)),
                           psumA, 6)
                for dd in sorted(psumA.keys()):
                    tail(dd, psumA[dd], w1_sb, wr1_sb, b1_sb, H, xT_sb,
                         h1_part, relu=True, make_h1T=True, dbg=debug)

            nc_.gpsimd.collective_compute(
                "AllGather", mybir.AluOpType.bypass,
                replica_groups=[list(range(cfg.NC))],
                ins=[h1_part[:ND, :]], outs=[h1_full[:]])

            # ---- layer 2: SWDGE gathers from h1_full ----
            items2 = s2['items']
            gathers = s2['gathers']
            for gg in range(DG):
                msgs = {}
                for go in s2['g_by_grp'][gg]:
                    _, mm, gl, cb, _sb = gathers[go]
                    msg = ms2p.tile([128, (gl // 128) * F], F16, tag="m2")
                    nc_.gpsimd.dma_gather(
                        out_ap=msg[:].rearrange("p (c e) -> p c e", e=F),
                        in_ap=h1_full[mm * CHS:(mm + 1) * CHS, :],
                        idxs_ap=idx_sb[:, cb:cb + gl // 16],
                        num_idxs=gl, num_idxs_reg=gl, elem_size=F,
                        queue_num=qctr[0] % 4)
                    qctr[0] += 1
                    msgs[go] = msg
                psumA = {}
                emit_items(msgs, items2,
                           iseg2_sb,
                           sorted(it2_by_grp[gg],
                                  key=lambda i: (items2[i][3], i)),
                           psumA, 4)
                for dd in sorted(psumA.keys()):
                    tail(dd, psumA[dd], w2_sb, wr2_sb, b2_sb, EMB, h1T_sb,
                         nd_part, relu=False, make_h1T=False, dbg=False,
                         odt=F32)

            # ---- final pair MLP ----
            zrow = wp.tile([1, EMB], F32, tag="zrow")
            nc_.vector.memset(zrow[:], 0.0)
            nc_.sync.dma_start(out=nd_part[NDP:NDP + 1, :], in_=zrow[:1, :])
            nfall = gp.tile([128, (B // 128) * EMB], F32, tag="nfall")
            nc_.gpsimd.dma_gather(
                out_ap=nfall[:].rearrange("p (c e) -> p c e", e=EMB),
                in_ap=nd_part[:, :], idxs_ap=nest_sb[:, :B // 16],
                num_idxs=B, num_idxs_reg=B, elem_size=EMB, queue_num=0)
            ffall = gp.tile([128, (B // 128) * EMB], F32, tag="ffall")
            nc_.gpsimd.dma_gather(
                out_ap=ffall[:].rearrange("p (c e) -> p c e", e=EMB),
                in_ap=nd_part[:, :], idxs_ap=food_sb[:, :B // 16],
                num_idxs=B, num_idxs_reg=B, elem_size=EMB, queue_num=1)
            for pt in range(B // 128):
                nf = wp.tile([128, EMB], F16, tag="nf")
                nc_.vector.tensor_copy(nf[:], nfall[:, pt * EMB:(pt + 1) * EMB])
                ff = wp.tile([128, EMB], F16, tag="ff")
                nc_.vector.tensor_copy(ff[:], ffall[:, pt * EMB:(pt + 1) * EMB])
                psumC = pcp.tile([128, 128], F16, tag="C", space="PSUM")
                nc_.tensor.transpose(out=psumC[:EMB, :], in_=nf[:],
                                     identity=ident[:])
                nfT = wp.tile([EMB, 128], F16, tag="nfT")
                nc_.vector.tensor_copy(nfT[:], psumC[:EMB, :])
                psumC2 = pcp.tile([128, 128], F16, tag="C", space="PSUM")
                nc_.tensor.transpose(out=psumC2[:EMB, :], in_=ff[:],
                                     identity=ident[:])
                ffT = wp.tile([EMB, 128], F16, tag="ffT")
                nc_.vector.tensor_copy(ffT[:], psumC2[:EMB, :])
                psumD = pbp.tile([128, H], F32, tag="B", space="PSUM")
                nc_.tensor.matmul(out=psumD[:], lhsT=nfT[:], rhs=fca_sb[:],
                                  start=True, stop=False)
                nc_.tensor.matmul(out=psumD[:], lhsT=ffT[:], rhs=fcb_sb[:],
                                  start=False, stop=False)
                nc_.tensor.matmul(out=psumD[:], lhsT=ones1[:1, :],
                                  rhs=fbias_sb[:1, :], start=False, stop=True)
                fo = wp.tile([128, H], F32, tag="fo")
                nc_.vector.tensor_copy(fo[:], psumD[:])
                nc_.sync.dma_start(out=cc_fin[pt * 128:(pt + 1) * 128, :],
                                   in_=fo[:])
            nc_.gpsimd.collective_compute(
                "AllReduce", mybir.AluOpType.add,
                replica_groups=[list(range(cfg.NC))],
                ins=[cc_fin[:]], outs=[cc_fin_o[:]])
            for pt in range(B // 128):
                ft = wp.tile([128, H], F32, tag="ft")
                nc_.sync.dma_start(out=ft[:],
                                   in_=cc_fin_o[pt * 128:(pt + 1) * 128, :])
                fo2 = wp.tile([128, H], F32, tag="fo2")
                nc_.scalar.activation(out=fo2[:], in_=ft[:],
                                      func=mybir.ActivationFunctionType.Tanh)
                nc_.sync.dma_start(out=t_out[pt * 128:(pt + 1) * 128, :],
                                   in_=fo2[:])
    nc_.compile()
    return nc_


def kernel(x, edge_src, edge_dst, edge_type, edge_attr, nest, food,
           W_rel1, W_root1, b1, W_rel2, W_root2, b2, fc_W, fc_b,
           _cfg=FULL, _runner=None, _debug=False):
    cfg = _cfg
    x = np.asarray(x, np.float32)
    src, dst, rel, core, dloc, recip_tab = _common(
        cfg, np.asarray(edge_src), np.asarray(edge_dst), np.asarray(edge_type))
    s1 = preprocess_l1(cfg, src, rel, core, dloc)
    s2 = preprocess_l2(cfg, src, rel, core, dloc)
    rect_rep = make_rect(cfg, recip_tab)
    nc_ = build(cfg, s1, s2, debug=_debug)

    x16 = x.astype(np.float16)
    W_rel1 = np.asarray(W_rel1, np.float32)
    W_rel2 = np.asarray(W_rel2, np.float32)
    w1 = W_rel1.transpose(1, 0, 2).reshape(cfg.F, cfg.R * cfg.H).astype(np.float16)
    w2 = W_rel2.transpose(1, 0, 2).reshape(cfg.H, cfg.R * cfg.EMB).astype(np.float16)
    fc_W = np.asarray(fc_W, np.float32)
    nest = np.asarray(nest, np.int64)
    food = np.asarray(food, np.int64)

    xT = x16.T
    in_maps = []
    for c in range(cfg.NC):
        # host-side layer-1 gather into stream order; pad slots point at
        # row 0 (their seg=-1 one-hot column masks them to zero anyway)
        xs = x16[s1['srcrow'][c]]
        xTc = np.zeros((cfg.F, cfg.NDP), np.float16)
        xTc[:, :cfg.ND] = xT[:, c * cfg.ND:(c + 1) * cfg.ND]
        nl = nest - c * cfg.ND
        nest_c = np.where((nl >= 0) & (nl < cfg.ND), nl, cfg.NDP).astype(np.int32)
        fl = food - c * cfg.ND
        food_c = np.where((fl >= 0) & (fl < cfg.ND), fl, cfg.NDP).astype(np.int32)
        in_maps.append(dict(
            xs=xs, xTc=xTc, idxw=s2['idx_rep'][c],
            iseg1=s1['iseg'][c], iseg2=s2['iseg'][c],
            rect=np.ascontiguousarray(rect_rep[c]),
            w1=w1, wr1=np.asarray(W_root1, np.float16),
            b1=np.asarray(b1, np.float16).reshape(1, -1),
            w2=w2, wr2=np.asarray(W_root2, np.float16),
            b2=np.asarray(b2, np.float16).reshape(1, -1),
            fca=fc_W[:cfg.EMB].astype(np.float16),
            fcb=fc_W[cfg.EMB:].astype(np.float16),
            fbias=(np.asarray(fc_b, np.float32).reshape(1, -1) / cfg.NC
                   ).astype(np.float16),
            nestw=np.tile(nest_c.astype(np.int16).reshape(-1, 16).T, (8, 1)),
            foodw=np.tile(food_c.astype(np.int16).reshape(-1, 16).T, (8, 1)),
        ))
    runner = _runner or (lambda n, im: run_bass_kernel_spmd(
        n, im, core_ids=list(range(cfg.NC))))
    res = runner(nc_, in_maps)
    return res.results[0]["out"]
